# revision 28
# baseline (speedup 1.0000x reference)
"""BinaryConv2d (3x3, SAME, NHWC) Trainium2 Bass kernel.

Strategy:
  - Data-parallel over batch: 32 images -> 8 cores x 4 images. Weights/bias
    replicated. No collectives needed.
  - Host prep (tiny): Wq = sign(W) cast to bf16 (+-1 exact), laid out as
    [cin, 9, cout]; bias replicated to [128, cout] f32.
  - Per core, per image (pipelined in 16-row chunks; casts are explicitly
    paced behind transposes so the SDMA fabric never starves the PE's
    critical path):
      1. SWDGE cast-DMA: x rows f32 NHWC (HBM) -> bf16 [16, W+2, cin] HBM
         scratch slices; pad columns 0 and W+1 zeroed from a zero SBUF tile
         (left/right SAME pads).
      2. HWDGE xbar transpose-DMA per chunk: scratch [(16*(W+2)), cin] ->
         SBUF chunk tile [cin, 16*(W+2)], channel-major.
      3. For each output row r: accumulate 9 (clipped at top/bottom) matmuls
         into PSUM [W, cout]: lhsT = chunk[:, (r+dh-1)%16*(W+2)+dw : +W]
         (stationary, pixels on PSUM partitions), rhs = Wq[:, 3*dh+dw, :]
         (streaming, cout free dim). fp32 PSUM accumulation, bf16 operands
         (rel err ~1.7e-3 vs the f32 reference).
      4. DVE tensor_add(psum, bias) -> SBUF f32 staging [W, 4, cout]; one
         HWDGE DMA per 4 rows out to NHWC HBM (keeps HWDGE op count low --
         per-DMA issue cost is ~0.7us and queue-pacing semaphores couple
         all HWDGE queues).

Measured on 8 axon-tunneled TRN2 cores: ~469 us HW exec, matmul stream at
~110 ns per N=256 matmul (2.4 GHz warm, LDWEIGHTS fully hidden).
"""

import numpy as np

N_CORES = 8
H = 112
W_DIM = 112
CIN = 128
COUT = 256
BATCH = 32
IMG_PER_CORE = BATCH // N_CORES


def _build_program(n_img, h, w, cin, cout):
    import bass_rust
    import concourse.bacc as bacc
    import concourse.mybir as mybir
    import concourse.tile as tile

    f32 = mybir.dt.float32
    bf16 = mybir.dt.bfloat16

    nc = bacc.Bacc(
        "TRN2", target_bir_lowering=False, debug=False, num_devices=N_CORES
    )
    x_d = nc.dram_tensor("x", [n_img, h, w, cin], f32, kind="ExternalInput").ap()
    w_d = nc.dram_tensor("w", [cin, 9, cout], bf16, kind="ExternalInput").ap()
    b_d = nc.dram_tensor("b", [128, cout], f32, kind="ExternalInput").ap()
    out_d = nc.dram_tensor(
        "out", [n_img, h, w, cout], f32, kind="ExternalOutput"
    ).ap()

    wp = w + 2  # padded row width in the transposed SBUF image
    # cast/transpose chunk row-counts; (sz * wp) % 16 == 0 required for the
    # xbar transpose, so sz % 8 == 0. Small first chunk shortens the
    # startup critical path (preamble -> cast0 -> transpose0 -> first MM).
    if h >= 32 and h % 16 == 0:
        sizes = [8] + [16] * ((h - 16) // 16) + [8]
    else:
        sizes = [16] * (h // 16)
    assert sum(sizes) == h and all((sz * wp) % 16 == 0 for sz in sizes)
    starts = [sum(sizes[:i]) for i in range(len(sizes))]
    n_chunks = len(sizes)
    rowmap = {}
    for ci, (s0, sz) in enumerate(zip(starts, sizes)):
        for i in range(s0, s0 + sz):
            rowmap[i] = (ci, i - s0)
    ob = 4  # output rows batched per store DMA
    assert h % ob == 0

    with tile.TileContext(nc) as tc:
        with (
            tc.tile_pool(name="consts", bufs=1) as cpool,
            tc.tile_pool(name="scratch", bufs=n_img, space="DRAM") as dpool,
            tc.tile_pool(name="xt", bufs=n_img * n_chunks) as xtpool,
            tc.tile_pool(name="psum", bufs=8, space="PSUM") as pspool,
            tc.tile_pool(name="outs", bufs=8) as opool,
        ):
            w_t = cpool.tile([cin, 9, cout], bf16)
            nc.sync.dma_start(out=w_t[:], in_=w_d[:])
            b_t = cpool.tile([128, cout], f32)
            nc.sync.dma_start(out=b_t[:], in_=b_d[:])
            zt = cpool.tile([h, cin], bf16)
            nc.vector.memset(zt[:], 0.0)

            # per-image DRAM scratch [h, wp, cin]; pad cols zeroed once per
            # image (SWDGE, keeping the HWDGE queue free for transposes);
            # casts fill 16-row slices; transposes lift 16-row slices to
            # SBUF chunk tiles [cin, rc*wp], channel-major.
            chunks = [[None] * n_chunks for _ in range(n_img)]
            transpose_insts = []
            PACE = 3  # cast for chunk g waits on transpose g-PACE: keeps the
            # SDMA fabric from flooding with casts and starving the
            # transposes the PE is actually waiting for

            def prep_image(img):
                scr = dpool.tile([h, wp, cin], bf16, tag="scr")
                for c, (r0, sz) in enumerate(zip(starts, sizes)):
                    # f32 -> bf16 cast during DMA (SWDGE only)
                    cast = nc.gpsimd.dma_start(
                        out=scr[r0 : r0 + sz, 1 : w + 1, :],
                        in_=x_d[img, r0 : r0 + sz],
                    )
                    if c == 0:
                        # pad-col zeroing rides behind the first cast so the
                        # critical-path cast issues immediately
                        nc.gpsimd.dma_start(out=scr[:, 0, :], in_=zt[:])
                        nc.gpsimd.dma_start(out=scr[:, wp - 1, :], in_=zt[:])
                    g = len(transpose_insts)
                    if g >= PACE:
                        bass_rust.add_dep_helper(
                            cast.ins,
                            transpose_insts[g - PACE].ins,
                            sync=True,
                            reason="pace casts behind transposes",
                        )
                    xt = xtpool.tile([cin, sz * wp], bf16, tag="xt")
                    tr = nc.sync.dma_start(
                        out=xt[:],
                        in_=scr[r0 : r0 + sz].rearrange("a b c -> (a b) c"),
                        transpose=True,
                    )
                    transpose_insts.append(tr)
                    chunks[img][c] = xt

            def get_row(img, i):
                # lhsT base AP for input row i of image img
                ci, off = rowmap[i]
                return chunks[img][ci], off * wp

            # issue ALL input prep up front: per-chunk region deps let
            # matmuls start as soon as chunk 0 is transposed, while the rest
            # streams in behind.
            for img in range(n_img):
                prep_image(img)

            for img in range(n_img):
                for rb in range(h // ob):
                    ot = opool.tile([w, ob, cout], f32)
                    for j in range(ob):
                        r = rb * ob + j
                        ps = pspool.tile([w, cout], f32)
                        taps = [
                            (dh, dw)
                            for dh in (0, 1, 2)
                            for dw in (0, 1, 2)
                            if 0 <= r + dh - 1 < h
                        ]
                        last = len(taps) - 1
                        for k, (dh, dw) in enumerate(taps):
                            xt, base = get_row(img, r + dh - 1)
                            nc.tensor.matmul(
                                ps[:],
                                xt[:, base + dw : base + dw + w],
                                w_t[:, 3 * dh + dw, :],
                                start=(k == 0),
                                stop=(k == last),
                            )
                        nc.vector.tensor_add(ot[:, j, :], ps[:], b_t[:w, :])
                    nc.scalar.dma_start(
                        out=out_d[img, rb * ob : (rb + 1) * ob].rearrange(
                            "j w c -> w j c"
                        ),
                        in_=ot[:],
                    )

    nc.compile()
    return nc


_cached_nc = None


def _get_program():
    global _cached_nc
    if _cached_nc is None:
        _cached_nc = _build_program(IMG_PER_CORE, H, W_DIM, CIN, COUT)
    return _cached_nc


def _prep_inputs(x, W, b):
    import ml_dtypes

    # sign with sign(0)=0, matching jnp.sign; bf16 holds +-1/0 exactly
    wq = np.sign(W.astype(np.float32)).astype(ml_dtypes.bfloat16)
    # [3,3,cin,cout] -> [cin, 9, cout]
    wq = np.ascontiguousarray(wq.transpose(2, 0, 1, 3).reshape(CIN, 9, COUT))
    b_rep = np.ascontiguousarray(
        np.broadcast_to(b.astype(np.float32), (128, COUT))
    )
    in_maps = []
    for c in range(N_CORES):
        xs = np.ascontiguousarray(
            x[c * IMG_PER_CORE : (c + 1) * IMG_PER_CORE].astype(np.float32)
        )
        in_maps.append({"x": xs, "w": wq, "b": b_rep})
    return in_maps


def run(x, W, b, trace=False, tmpdir=None):
    from concourse import bass_utils

    if trace:
        # the agent image's antenv lacks axon_hooks; wire the NTFF profile
        # hook up manually so trace=True yields exec_time_ns + pftrace
        import sys, types

        if "antenv.axon_hooks" not in sys.modules:
            import antenv
            from trn_agent_boot.trn_boot import _ntff_profile_via_ctypes

            mod = types.ModuleType("antenv.axon_hooks")
            _hook = _ntff_profile_via_ctypes("/opt/axon/libaxon_pjrt.so")
            mod.get_axon_ntff_profile_hook = lambda: _hook
            sys.modules["antenv.axon_hooks"] = mod
            antenv.axon_hooks = mod

    nc = _get_program()
    in_maps = _prep_inputs(x, W, b)
    res = bass_utils.run_bass_kernel_spmd(
        nc, in_maps, list(range(N_CORES)), trace=trace, tmpdir=tmpdir
    )
    out = np.concatenate([res.results[i]["out"] for i in range(N_CORES)], axis=0)
    return out, res


def kernel(x, W, b):
    out, _ = run(x, W, b, trace=False)
    return out


# revision 29
# speedup vs baseline: 1.0184x; 1.0184x over previous
"""BinaryConv2d (3x3, SAME, NHWC) Trainium2 Bass kernel.

Strategy:
  - Data-parallel over batch: 32 images -> 8 cores x 4 images. Weights/bias
    replicated. No collectives needed.
  - Host prep (tiny): Wq = sign(W) cast to bf16 (+-1 exact), laid out as
    [cin, 9, cout]; bias replicated to [128, cout] f32.
  - Per core, per image:
      1. SWDGE cast-DMA: x[img] f32 NHWC (HBM) -> bf16 [H, W+2, cin] HBM
         scratch, middle columns; columns 0 and W+1 are zeroed by two small
         DMAs from a zero SBUF tile (left/right SAME pads).
      2. HWDGE xbar transpose-DMA: scratch [(H*(W+2)), cin] -> SBUF
         xT [cin, H*(W+2)] contiguous, channel-major.
      3. For each output row r: accumulate 9 (clipped at top/bottom) matmuls
         into PSUM [W, cout]: lhsT = xT[:, (r+dh-1)*(W+2)+dw : +W]
         (stationary, pixels on PSUM partitions), rhs = Wq[:, 3*dh+dw, :]
         (streaming, cout free dim). fp32 PSUM accumulation.
      4. DVE tensor_add(psum, bias) -> SBUF f32, HWDGE DMA out to NHWC HBM.
"""

import numpy as np

N_CORES = 8
H = 112
W_DIM = 112
CIN = 128
COUT = 256
BATCH = 32
IMG_PER_CORE = BATCH // N_CORES


def _build_program(n_img, h, w, cin, cout):
    import bass_rust
    import concourse.bacc as bacc
    import concourse.mybir as mybir
    import concourse.tile as tile

    f32 = mybir.dt.float32
    bf16 = mybir.dt.bfloat16

    nc = bacc.Bacc(
        "TRN2", target_bir_lowering=False, debug=False, num_devices=N_CORES
    )
    x_d = nc.dram_tensor("x", [n_img, h, w, cin], f32, kind="ExternalInput").ap()
    w_d = nc.dram_tensor("w", [cin, 9, cout], bf16, kind="ExternalInput").ap()
    b_d = nc.dram_tensor("b", [128, cout], f32, kind="ExternalInput").ap()
    out_d = nc.dram_tensor(
        "out", [n_img, h, w, cout], f32, kind="ExternalOutput"
    ).ap()

    wp = w + 2  # padded row width in the transposed SBUF image
    rc = 16  # rows per cast/transpose chunk; (rc * wp) % 16 == 0 required
    assert h % rc == 0 and (rc * wp) % 16 == 0
    n_chunks = h // rc
    ob = 4  # output rows batched per store DMA
    assert h % ob == 0

    with tile.TileContext(nc) as tc:
        with (
            tc.tile_pool(name="consts", bufs=1) as cpool,
            tc.tile_pool(name="scratch", bufs=n_img, space="DRAM") as dpool,
            tc.tile_pool(name="xt", bufs=n_img * n_chunks) as xtpool,
            tc.tile_pool(name="psum", bufs=8, space="PSUM") as pspool,
            tc.tile_pool(name="outs", bufs=8) as opool,
        ):
            w_t = cpool.tile([cin, 9, cout], bf16)
            nc.sync.dma_start(out=w_t[:], in_=w_d[:])
            b_t = cpool.tile([128, cout], f32)
            nc.sync.dma_start(out=b_t[:], in_=b_d[:])
            zt = cpool.tile([h, cin], bf16)
            nc.vector.memset(zt[:], 0.0)

            # per-image DRAM scratch [h, wp, cin]; pad cols zeroed once per
            # image (SWDGE, keeping the HWDGE queue free for transposes);
            # casts fill 16-row slices; transposes lift 16-row slices to
            # SBUF chunk tiles [cin, rc*wp], channel-major.
            chunks = [[None] * n_chunks for _ in range(n_img)]
            transpose_insts = []
            PACE = 3  # cast for chunk g waits on transpose g-PACE: keeps the
            # SDMA fabric from flooding with casts and starving the
            # transposes the PE is actually waiting for

            def prep_image(img):
                scr = dpool.tile([h, wp, cin], bf16, tag="scr")
                for c in range(n_chunks):
                    r0 = c * rc
                    # f32 -> bf16 cast during DMA (SWDGE only)
                    cast = nc.gpsimd.dma_start(
                        out=scr[r0 : r0 + rc, 1 : w + 1, :],
                        in_=x_d[img, r0 : r0 + rc],
                    )
                    if c == 0:
                        # pad-col zeroing rides behind the first cast so the
                        # critical-path cast issues immediately
                        nc.gpsimd.dma_start(out=scr[:, 0, :], in_=zt[:])
                        nc.gpsimd.dma_start(out=scr[:, wp - 1, :], in_=zt[:])
                    g = len(transpose_insts)
                    if g >= PACE:
                        bass_rust.add_dep_helper(
                            cast.ins,
                            transpose_insts[g - PACE].ins,
                            sync=True,
                            reason="pace casts behind transposes",
                        )
                    xt = xtpool.tile([cin, rc * wp], bf16, tag="xt")
                    tr = nc.sync.dma_start(
                        out=xt[:],
                        in_=scr[r0 : r0 + rc].rearrange("a b c -> (a b) c"),
                        transpose=True,
                    )
                    transpose_insts.append(tr)
                    chunks[img][c] = xt

            def get_row(img, i):
                # lhsT base AP for input row i of image img
                return chunks[img][i // rc], (i % rc) * wp

            # issue ALL input prep up front: per-chunk region deps let
            # matmuls start as soon as chunk 0 is transposed, while the rest
            # streams in behind.
            for img in range(n_img):
                prep_image(img)

            for img in range(n_img):
                for rb in range(h // ob):
                    ot = opool.tile([w, ob, cout], f32)
                    for j in range(ob):
                        r = rb * ob + j
                        ps = pspool.tile([w, cout], f32)
                        taps = [
                            (dh, dw)
                            for dh in (0, 1, 2)
                            for dw in (0, 1, 2)
                            if 0 <= r + dh - 1 < h
                        ]
                        last = len(taps) - 1
                        for k, (dh, dw) in enumerate(taps):
                            xt, base = get_row(img, r + dh - 1)
                            nc.tensor.matmul(
                                ps[:],
                                xt[:, base + dw : base + dw + w],
                                w_t[:, 3 * dh + dw, :],
                                start=(k == 0),
                                stop=(k == last),
                            )
                        nc.vector.tensor_add(ot[:, j, :], ps[:], b_t[:w, :])
                    nc.scalar.dma_start(
                        out=out_d[img, rb * ob : (rb + 1) * ob].rearrange(
                            "j w c -> w j c"
                        ),
                        in_=ot[:],
                    )

    nc.compile()
    return nc


_cached_nc = None


def _get_program():
    global _cached_nc
    if _cached_nc is None:
        _cached_nc = _build_program(IMG_PER_CORE, H, W_DIM, CIN, COUT)
    return _cached_nc


def _prep_inputs(x, W, b):
    import ml_dtypes

    # sign with sign(0)=0, matching jnp.sign; bf16 holds +-1/0 exactly
    wq = np.sign(W.astype(np.float32)).astype(ml_dtypes.bfloat16)
    # [3,3,cin,cout] -> [cin, 9, cout]
    wq = np.ascontiguousarray(wq.transpose(2, 0, 1, 3).reshape(CIN, 9, COUT))
    b_rep = np.ascontiguousarray(
        np.broadcast_to(b.astype(np.float32), (128, COUT))
    )
    in_maps = []
    for c in range(N_CORES):
        xs = np.ascontiguousarray(
            x[c * IMG_PER_CORE : (c + 1) * IMG_PER_CORE].astype(np.float32)
        )
        in_maps.append({"x": xs, "w": wq, "b": b_rep})
    return in_maps


def run(x, W, b, trace=False, tmpdir=None):
    from concourse import bass_utils

    if trace:
        # the agent image's antenv lacks axon_hooks; wire the NTFF profile
        # hook up manually so trace=True yields exec_time_ns + pftrace
        import sys, types

        if "antenv.axon_hooks" not in sys.modules:
            import antenv
            from trn_agent_boot.trn_boot import _ntff_profile_via_ctypes

            mod = types.ModuleType("antenv.axon_hooks")
            _hook = _ntff_profile_via_ctypes("/opt/axon/libaxon_pjrt.so")
            mod.get_axon_ntff_profile_hook = lambda: _hook
            sys.modules["antenv.axon_hooks"] = mod
            antenv.axon_hooks = mod

    nc = _get_program()
    in_maps = _prep_inputs(x, W, b)
    res = bass_utils.run_bass_kernel_spmd(
        nc, in_maps, list(range(N_CORES)), trace=trace, tmpdir=tmpdir
    )
    out = np.concatenate([res.results[i]["out"] for i in range(N_CORES)], axis=0)
    return out, res


def kernel(x, W, b):
    out, _ = run(x, W, b, trace=False)
    return out
